# revision 40
# baseline (speedup 1.0000x reference)
"""Trainium2 Bass kernel for nn_AtomUpdateBlock (GemNet AtomUpdateBlock).

Computation (see reference):
    mlp_rbf = rbf @ W_rbf.T            # [E, de]
    x = m * mlp_rbf                    # [E, de]
    x2 = segment_sum(x, id_j, nAtoms)  # [nAtoms, de]
    x = scaled_silu(x2*scale @ W1.T); 2x residual layers; out [nAtoms, da]

Strategy: atom-shard across the 8 cores (12500 atoms each). Host sorts edges
by target atom (argsort) and hands each core the m-rows / rbf-rows of exactly
its own edges, grouped into 128-atom windows and padded to whole 128-edge
tiles. Each 128-edge tile computes x = m * (rbf @ W_rbf.T), then scatter-adds
its edges into the window's PSUM accumulator with a one-hot matmul.

V3 structure (per group of 16 tiles):
 - one-hot built ON DEVICE: host ships one fp16 window-column index per edge
   (cols stream, ~0.3MB/core); tensor_scalar(is_equal, iota_const, col_vec)
   builds each [128,128] one-hot tile, split across GPSIMD + DVE.
 - rbf matmuls: 2 tiles stacked per 32-row strip (rows 32s+16a..+16), 4
   strips at tile_position (32s, 0) -> 4-way concurrent small matmuls with a
   dense [128, 256] rbf DMA per group.
 - mlp_rbf PSUM -> SBUF fp16 cast on ACT, then one [128,2048] fp16 DVE
   tensor_tensor multiply with m (2x mode).
 - scatter matmul per tile: lhsT = xt (fp16, FWL), rhs = one-hot, N=128.
 - MLP per 500-atom slice: 5 matmul+silu, final (a*tt + b*s5) computed on the
   PE as two scaled-identity matmuls accumulated in PSUM, output DMA'd
   straight from PSUM.
"""

import os
import sys
import time
from contextlib import ExitStack

sys.path.insert(0, "/opt/trn_rl_repo")

import ml_dtypes
import numpy as np

NCORES = 8
E = 1_000_000
NATOMS = 100_000
DE = 128
DRBF = 16
P = 128          # edges per tile
WND = 128        # atoms per phase-1 window
GRP = 16         # tiles per group (batched DMA / DVE ops)
COLCH = 64       # tiles per cols DMA chunk
CAST_ACT_N = 0   # of every 8 groups, this many use ACT-cast + fp16 2x TT mult
MLPW = 448       # acc columns per phase-2 (MLP) slice
NGW = (NATOMS + WND - 1) // WND    # 782 global 128-atom windows
NW = (NGW + NCORES - 1) // NCORES  # 98 window slots per core
A_CORE = NW * WND                  # 12544 acc columns per core
NMLP = A_CORE // MLPW              # 28 phase-2 slices per core
PAD_COL = 4096.0                   # one-hot column id that never matches
ACT_FN = "Silu"                    # sim override: CoreSim lacks Silu

INV_SCALE_SILU = 1.0 / 0.6
INV_SQRT2 = 2.0 ** -0.5

_PROGRAM_CACHE: dict = {}


def _build_program(t_list, wd_list, epad, ntiles):
    import concourse.bacc as bacc
    import concourse.mybir as mybir
    import concourse.tile as tile

    dt = mybir.dt
    op = mybir.AluOpType
    act = mybir.ActivationFunctionType

    nc = bacc.Bacc(
        "TRN2", target_bir_lowering=False, debug=False, num_devices=NCORES
    )

    ngrp = ntiles // GRP
    nch = (ntiles + COLCH - 1) // COLCH

    m_pad = nc.dram_tensor("m_pad", [ngrp * P, GRP * DE], dt.float16, kind="ExternalInput").ap()
    rbf4 = nc.dram_tensor("rbf4", [ngrp * P, 2 * P], dt.float16, kind="ExternalInput").ap()
    oh_in = nc.dram_tensor("oh_in", [ngrp * P, GRP * WND], dt.float8e4, kind="ExternalInput").ap()
    wrbf_stk_in = nc.dram_tensor("wrbf_stk_in", [P, 8 * DE], dt.float16, kind="ExternalInput").ap()
    ident_in = nc.dram_tensor("ident_in", [P, 2 * P], dt.float16, kind="ExternalInput").ap()
    wmlp_in = [
        nc.dram_tensor(f"wmlp{i}_in", [DE, DE], dt.float16, kind="ExternalInput").ap()
        for i in range(5)
    ]
    out = nc.dram_tensor("out", [DE, A_CORE], dt.float16, kind="ExternalOutput").ap()

    # window bookkeeping: first/last tile of each window
    w_start = []
    w_end = []
    pos = 0
    for t_w in t_list:
        w_start.append(pos)
        w_end.append(pos + t_w - 1)
        pos += t_w
    assert pos == ntiles and ntiles % GRP == 0
    tile_window = np.repeat(np.arange(NW), t_list)

    with tile.TileContext(nc) as tc, ExitStack() as ctx:
        const_p = ctx.enter_context(tc.tile_pool(name="const_p", bufs=1))
        acc_sb_p = ctx.enter_context(tc.tile_pool(name="acc_sb_p", bufs=1))
        m_p = ctx.enter_context(tc.tile_pool(name="m_p", bufs=6))
        rbfq_p = ctx.enter_context(tc.tile_pool(name="rbfq_p", bufs=5))
        rbf_sb_p = ctx.enter_context(tc.tile_pool(name="rbf_sb_p", bufs=2))
        x_p = ctx.enter_context(tc.tile_pool(name="x_p", bufs=5))
        oh_p = ctx.enter_context(tc.tile_pool(name="oh_p", bufs=5))
        mlp_ps_p = ctx.enter_context(tc.tile_pool(name="mlp_ps_p", bufs=2, space="PSUM"))
        acc_ps_p = ctx.enter_context(tc.tile_pool(name="acc_ps_p", bufs=2, space="PSUM"))
        z_ps_p = ctx.enter_context(tc.tile_pool(name="z_ps_p", bufs=2, space="PSUM"))
        s_p = ctx.enter_context(tc.tile_pool(name="s_p", bufs=3))

        # load constants once
        wrbf_stk = const_p.tile([P, 8 * DE], dt.float16)
        nc.scalar.dma_start(wrbf_stk[:], wrbf_stk_in[:])
        ident_sb = const_p.tile([P, 2 * P], dt.float16, name="ident_sb")
        nc.scalar.dma_start(ident_sb[:], ident_in[:])
        wmlp_sb = []
        for i in range(5):
            wt = const_p.tile([DE, DE], dt.float16, name=f"wmlp_sb{i}")
            nc.scalar.dma_start(wt[:], wmlp_in[i][:])
            wmlp_sb.append(wt)

        acc_sb = acc_sb_p.tile([P, A_CORE], dt.float16)

        # ---- phase 2 (interleaved): MLP slice s once its windows flushed ----
        def emit_mlp(s):
            sl = slice(s * MLPW, (s + 1) * MLPW)
            z_ps = z_ps_p.tile([P, MLPW], dt.float32, tag="z_ps")
            nc.tensor.matmul(out=z_ps[:], lhsT=wmlp_sb[0][:], rhs=acc_sb[:, sl],
                             start=True, stop=True)
            s1 = s_p.tile([P, MLPW], dt.float16, tag="s1")
            nc.scalar.activation(s1[:], z_ps[:], act.Silu)

            u_ps = z_ps_p.tile([P, MLPW], dt.float32, tag="z_ps")
            nc.tensor.matmul(out=u_ps[:], lhsT=wmlp_sb[1][:], rhs=s1[:],
                             start=True, stop=True)
            s2 = s_p.tile([P, MLPW], dt.float16, tag="s2")
            nc.scalar.activation(s2[:], u_ps[:], act.Silu)

            u2_ps = z_ps_p.tile([P, MLPW], dt.float32, tag="z_ps")
            nc.tensor.matmul(out=u2_ps[:], lhsT=wmlp_sb[2][:], rhs=s2[:],
                             start=True, stop=True)
            s3 = s_p.tile([P, MLPW], dt.float16, tag="s2")
            nc.scalar.activation(s3[:], u2_ps[:], act.Silu)

            tt = s_p.tile([P, MLPW], dt.float16, tag="tt")
            nc.vector.tensor_tensor(out=tt[:], in0=s1[:], in1=s3[:], op=op.add)

            u3_ps = z_ps_p.tile([P, MLPW], dt.float32, tag="z_ps")
            nc.tensor.matmul(out=u3_ps[:], lhsT=wmlp_sb[3][:], rhs=tt[:],
                             start=True, stop=True)
            s4 = s_p.tile([P, MLPW], dt.float16, tag="s1")
            nc.scalar.activation(s4[:], u3_ps[:], act.Silu)

            u4_ps = z_ps_p.tile([P, MLPW], dt.float32, tag="z_ps")
            nc.tensor.matmul(out=u4_ps[:], lhsT=wmlp_sb[4][:], rhs=s4[:],
                             start=True, stop=True)
            s5 = s_p.tile([P, MLPW], dt.float16, tag="s2")
            nc.scalar.activation(s5[:], u4_ps[:], act.Silu)

            # final combine on PE: out_ps = a*tt + b*s5 via scaled identities
            o_ps = z_ps_p.tile([P, MLPW], dt.float32, tag="z_ps")
            nc.tensor.matmul(out=o_ps[:], lhsT=ident_sb[:, :P], rhs=tt[:],
                             start=True, stop=False, skip_group_check=True)
            nc.tensor.matmul(out=o_ps[:], lhsT=ident_sb[:, P:], rhs=s5[:],
                             start=False, stop=True, skip_group_check=True)
            o_sb = s_p.tile([P, MLPW], dt.float16, tag="o_sb")
            nc.vector.tensor_copy(out=o_sb[:], in_=o_ps[:])
            nc.scalar.dma_start(out[:, sl], o_sb[:])

        mlp_after = {}
        for _s in range(NMLP):
            _w = -(-((_s + 1) * MLPW) // WND) - 1
            mlp_after.setdefault(_w, []).append(_s)

        acc_ps = None
        xt_hist = {}
        oh_hist = {}

        def emit_front(g):
            m4 = m_p.tile([P, GRP * DE], dt.float16, tag="m4")
            nc.sync.dma_start(m4[:], m_pad[g * P : (g + 1) * P, :])
            rbfq = rbfq_p.tile([P, 2 * P], dt.float16, tag="rbfq")
            nc.sync.dma_start(rbfq[:], rbf4[g * P : (g + 1) * P, :])
            oh4 = oh_p.tile([P, GRP * WND], dt.float8e4, tag="oh4")
            nc.gpsimd.dma_start(oh4[:], oh_in[g * P : (g + 1) * P, :])
            oh_hist[g] = oh4

            # rbf matmuls: 8 tiles stacked per [128,128] stationary (rows
            # 16k..16k+15 = tile k's rbf.T), rhs = zero-masked wrbf variants,
            # 4 tiles per N=512 matmul. Half-group -> [128, 1024] PSUM.
            # The x-multiply reads PSUM directly on DVE (1x) for most groups;
            # CAST_ACT_N of every 8 groups route through an ACT fp16 cast +
            # one 2x fp16 TT to offload DVE.
            xt4 = x_p.tile([P, GRP * DE], dt.float16, tag="xt4")
            act_mode = (g % 8) < CAST_ACT_N
            if act_mode:
                rbf_sb = rbf_sb_p.tile([P, GRP * DE], dt.float16, tag="rbf_sb")
            for q in range(2):
                mlp_ps = mlp_ps_p.tile([P, 8 * DE], dt.float32, tag="mlp_ps")
                for h in range(2):
                    nc.tensor.matmul(
                        out=mlp_ps[:, h * 4 * DE : (h + 1) * 4 * DE],
                        lhsT=rbfq[:, q * P : (q + 1) * P],
                        rhs=wrbf_stk[:, h * 4 * DE : (h + 1) * 4 * DE],
                        start=True, stop=True,
                        skip_group_check=True,
                    )
                if act_mode:
                    nc.scalar.activation(
                        rbf_sb[:, q * 8 * DE : (q + 1) * 8 * DE], mlp_ps[:],
                        act.Copy, bias=0.0, scale=1.0,
                    )
                else:
                    nc.vector.tensor_tensor(
                        out=xt4[:, q * 8 * DE : (q + 1) * 8 * DE],
                        in0=m4[:, q * 8 * DE : (q + 1) * 8 * DE],
                        in1=mlp_ps[:], op=op.mult,
                    )
            if act_mode:
                nc.vector.tensor_tensor(out=xt4[:], in0=m4[:], in1=rbf_sb[:], op=op.mult)
            xt_hist[g] = xt4

        def emit_back(g):
            nonlocal acc_ps
            xt4 = xt_hist.pop(g)
            oh4 = oh_hist.pop(g)
            for i in range(GRP):
                gt = g * GRP + i
                w = tile_window[gt]
                wd = wd_list[w]
                if gt == w_start[w]:
                    acc_ps = acc_ps_p.tile([P, WND], dt.float32, tag="acc_ps")
                nc.tensor.matmul(
                    out=acc_ps[:, :wd],
                    lhsT=xt4[:, i * DE : (i + 1) * DE],
                    rhs=oh4[:, i * WND : i * WND + wd],
                    start=(gt == w_start[w]),
                    stop=(gt == w_end[w]),
                    skip_group_check=True,
                )
                if gt == w_end[w]:
                    nc.vector.tensor_copy(
                        out=acc_sb[:, w * WND : w * WND + wd], in_=acc_ps[:, :wd]
                    )
                    for _s in mlp_after.get(w, []):
                        emit_mlp(_s)

        for g in range(ngrp + 2):
            if g < ngrp:
                emit_front(g)
            if g >= 2:
                emit_back(g - 2)

    nc.compile()
    return nc


def _prepare(m, rbf, id_j, W_rbf, scale, W1, W_res):
    """Host-side: sort edges by atom, assign global 128-atom windows to
    (core, slot) pairs balanced by edge count, bucket into padded tiles."""
    id_j = np.ascontiguousarray(np.asarray(id_j).astype(np.int64))
    perm = np.argsort(id_j, kind="stable")
    ids_sorted = id_j[perm]

    # global window edge ranges
    gw_bounds = np.minimum(np.arange(NGW + 1) * WND, NATOMS)
    gw_edges = np.searchsorted(ids_sorted, gw_bounds)  # [NGW+1]
    gw_cnt = np.diff(gw_edges)

    # balanced assignment: sort windows by count desc; slot k gets ranks
    # [8k, 8k+8) across the 8 cores -> max-within-slot ~= mean
    order = np.argsort(-gw_cnt, kind="stable")
    assign = np.full((NCORES, NW), -1, dtype=np.int64)  # global window id
    for k in range(NW):
        blk = order[k * NCORES : (k + 1) * NCORES]
        assign[: len(blk), k] = blk
    assign = assign[:, ::-1].copy()  # ascending sizes: big windows flush last

    counts = np.zeros((NCORES, NW), dtype=np.int64)
    for c in range(NCORES):
        for k in range(NW):
            w = assign[c, k]
            if w >= 0:
                counts[c, k] = gw_cnt[w]
    t_list = np.maximum(1, -(-counts.max(axis=0) // P)).astype(np.int64)  # [NW]
    rem = (-int(t_list.sum())) % GRP
    t_list[-1] += rem
    wd_list = [WND] * NW
    ntiles = int(t_list.sum())
    epad = ntiles * P

    gidx = np.zeros((NCORES, epad), dtype=np.int64)
    cols = np.full((NCORES, epad), PAD_COL, dtype=np.float32)
    # out_map[c, j] = global atom for core c's acc column j (-1 = unused)
    out_map = np.full((NCORES, A_CORE), -1, dtype=np.int64)
    for c in range(NCORES):
        pos = 0
        for k in range(NW):
            w = assign[c, k]
            if w >= 0:
                s0, e0 = gw_edges[w], gw_edges[w + 1]
                n = e0 - s0
                base = gw_bounds[w]
                na = gw_bounds[w + 1] - base
                out_map[c, k * WND : k * WND + na] = base + np.arange(na)
            else:
                s0 = e0 = 0
                n = 0
                base = 0
            gidx[c, pos : pos + n] = perm[s0:e0]
            if n < t_list[k] * P:
                gidx[c, pos + n : pos + t_list[k] * P] = perm[s0] if n > 0 else 0
            cols[c, pos : pos + n] = ids_sorted[s0:e0] - base
            pos += t_list[k] * P

    # constants / weights
    q = INV_SCALE_SILU
    c2 = INV_SQRT2
    a_const = q * 0.5
    b_const = q * c2
    scale = float(np.asarray(scale))
    wmlp_np = [
        np.ascontiguousarray((W1 * scale).T).astype(np.float16),
        np.ascontiguousarray((W_res[0, 0] * q).T).astype(np.float16),
        np.ascontiguousarray((W_res[0, 1] * q).T).astype(np.float16),
        np.ascontiguousarray((W_res[1, 0] * (q * c2)).T).astype(np.float16),
        np.ascontiguousarray((W_res[1, 1] * q).T).astype(np.float16),
    ]
    wrbf_t = np.ascontiguousarray(W_rbf.T).astype(np.float16)  # [16, 128]
    wrbf_stk = np.zeros((P, 8 * DE), dtype=np.float16)
    for k in range(8):
        wrbf_stk[16 * k : 16 * k + 16, k * DE : (k + 1) * DE] = wrbf_t
    ident_np = np.zeros((P, 2 * P), dtype=np.float16)
    ident_np[:, :P] = np.eye(P, dtype=np.float16) * np.float16(a_const)
    ident_np[:, P:] = np.eye(P, dtype=np.float16) * np.float16(b_const)

    ngrp = ntiles // GRP
    nch = (ntiles + COLCH - 1) // COLCH

    in_maps = []
    for c in range(NCORES):
        g = gidx[c]
        m_pad = np.ascontiguousarray(
            m[g].astype(np.float16).reshape(ngrp, GRP, P, DE)
            .transpose(0, 2, 1, 3).reshape(ngrp * P, GRP * DE)
        )  # grouped: row g*128+p = 16 tiles' row p
        # rbf4: [g, 16k+r, q*128+e] = rbf[edge(g, i=q*8+k, e), r]
        rbf_e = rbf[g].astype(np.float16).reshape(ngrp, GRP, P, DRBF)  # [g,i,e,r]
        rbf4 = np.zeros((ngrp, P, 2 * P), dtype=np.float16)
        for i in range(GRP):
            k, qq = i % 8, i // 8
            rbf4[:, 16 * k : 16 * k + 16,
                 qq * P : (qq + 1) * P] = rbf_e[:, i].transpose(0, 2, 1)
        rbf4 = np.ascontiguousarray(rbf4.reshape(ngrp * P, 2 * P))
        oh = (
            cols[c].astype(np.int32)[:, None] == np.arange(WND, dtype=np.int32)[None, :]
        ).astype(ml_dtypes.float8_e4m3fn).reshape(ngrp, GRP, P, WND)
        oh = np.ascontiguousarray(
            oh.transpose(0, 2, 1, 3).reshape(ngrp * P, GRP * WND)
        )
        im = {
            "m_pad": m_pad,
            "rbf4": rbf4,
            "oh_in": oh,
            "wrbf_stk_in": wrbf_stk,
            "ident_in": ident_np,
        }
        for i in range(5):
            im[f"wmlp{i}_in"] = wmlp_np[i]
        in_maps.append(im)

    return tuple(t_list.tolist()), tuple(wd_list), epad, ntiles, in_maps, out_map


def _run(inputs, trace=False):
    from concourse.bass_utils import run_bass_kernel_spmd

    nAtoms = int(np.asarray(inputs["nAtoms"]))
    assert nAtoms == NATOMS, f"kernel hardcoded for nAtoms={NATOMS}, got {nAtoms}"
    m = np.asarray(inputs["m"], dtype=np.float32)
    assert m.shape == (E, DE), m.shape

    t_list, wd_list, epad, ntiles, in_maps, out_maps = _prepare(
        m,
        np.asarray(inputs["rbf"], dtype=np.float32),
        inputs["id_j"],
        np.asarray(inputs["W_rbf"], dtype=np.float32),
        inputs["scale"],
        np.asarray(inputs["W1"], dtype=np.float32),
        np.asarray(inputs["W_res"], dtype=np.float32),
    )

    key = (t_list, epad)
    if key not in _PROGRAM_CACHE:
        _PROGRAM_CACHE.clear()
        _PROGRAM_CACHE[key] = _build_program(t_list, wd_list, epad, ntiles)
    nc = _PROGRAM_CACHE[key]

    res = run_bass_kernel_spmd(
        nc, in_maps, core_ids=list(range(NCORES)), trace=trace
    )
    out_full = np.empty((NATOMS, DE), dtype=np.float32)
    for c in range(NCORES):
        om = out_maps[c]
        sel = om >= 0
        out_full[om[sel]] = res.results[c]["out"].astype(np.float32).T[sel]
    return np.ascontiguousarray(out_full), res.exec_time_ns


def kernel(**inputs) -> np.ndarray:
    out, _ = _run(inputs, trace=False)
    return out
